# revision 19
# baseline (speedup 1.0000x reference)
"""MoE logistic regression kernel for 8 Trainium2 NeuronCores.

Math (after dead-code elimination of the reference's unused router path):
    noise_logits = x @ noise_w.T + noise_b            # [B, E]
    top8 = top_k(noise_logits, 8)
    gates = softmax over the top-8 entries (others 0)
    expert = sigmoid(x @ expert_w.T + expert_b)       # [B, E]
    out[b] = sum_e gates[b,e] * expert[b,e]           # [B, 1]

Sharding: batch split 8 ways (2048 rows/core); weights replicated.

Implementation notes:
- Single-pass fp16 matmul (x, w rounded on host). Logit error ~2.3e-4
  flips the 8th/9th expert on ~24/16384 rows; end-to-end l2 rel err
  ~1.2e-3 vs the 2e-2 gate, at half the DMA and a third of the PE work
  of an fp16 hi/lo split. The x stream is the roofline: ~46.6us of
  gapless DMA at the HBM limit.
- Batch-tile-major stream: each tile's full contraction arrives while
  the previous tile's epilogue runs on ACT/DVE. Tile widths taper
  (512,512,512,256,128,128) and the last tile's DMA groups taper too,
  so the serial tail after the last byte is one short epilogue.
- The epilogue never uses the ACT sigmoid table: sigmoid comes from
  exp(-z) + DVE 1/(1+e), and softmax skips the max-shift (logits are
  bounded ~|4|), so every ACT op stays in the one exp_and_others
  function set -- no mid-stream LoadActFuncSet (1.3us each).
- Top-8 gating via DVE Max8 + fused (e >= e8) mask * e with accumulated
  row sum (scalar_tensor_tensor), all on the SBUF exp(v) copy --
  exp is monotone so selection is identical, and avoiding a second
  PSUM reader dodges cross-engine read serialization.
- Per-tile outputs DMA straight from the [128, j] result (no final
  transpose); early tiles go via the idle gpsimd SWDGE path so they
  never head-of-line-block the x stream on the SP HWDGE queue.
"""

import sys

import numpy as np

if "/opt/trn_rl_repo" not in sys.path:
    sys.path.insert(0, "/opt/trn_rl_repo")

B, D, E, TOPK, NCORES = 16384, 4096, 64, 8, 8
BC = B // NCORES      # batch rows per core
NK = D // 128         # contraction chunks
TILES = [512, 512, 512, 256, 128, 128]          # batch tile widths
OFFS = [sum(TILES[:i]) for i in range(len(TILES))]
assert sum(TILES) == BC
# DMA grouping in k-chunks per tile; the final tile tapers so almost no
# matmul work remains after the last byte lands
GROUPS = [[8, 8, 8, 8]] * 5 + [[16, 8, 4, 4]]

_cached = {}


def _build_program():
    import concourse.bass as bass
    import concourse.tile as tile
    from concourse import bacc, mybir
    from concourse.masks import make_identity

    f32 = mybir.dt.float32
    f16 = mybir.dt.float16
    bf16 = mybir.dt.bfloat16
    act = mybir.ActivationFunctionType
    alu = mybir.AluOpType

    nc = bacc.Bacc("TRN2", target_bir_lowering=False, debug=False)
    # x fp16, per-tile partition-major blocks concatenated flat:
    # tile t occupies [128, NK, bt] at element offset 128*NK*OFFS[t], so
    # every group DMA is one contiguous gsz*bt*2-byte run per partition.
    xt = nc.dram_tensor("xt", [NK * 128 * BC], f16, kind="ExternalInput").ap()
    wt0 = nc.dram_tensor("wt0", [128, 8 * 128], f16, kind="ExternalInput").ap()
    wt1 = nc.dram_tensor("wt1", [128, (NK - 8) * 128], f16,
                         kind="ExternalInput").ap()
    bb = nc.dram_tensor("bb", [128, 1], f32, kind="ExternalInput").ap()
    out = nc.dram_tensor("out", [BC, 1], f32, kind="ExternalOutput").ap()

    with tile.TileContext(nc) as tc:
        with (
            tc.tile_pool(name="consts", bufs=1) as consts,
            tc.tile_pool(name="xpool", bufs=6) as xpool,
            tc.tile_pool(name="eppool", bufs=4) as eppool,
            tc.tile_pool(name="small", bufs=2) as small,
            tc.tile_pool(name="tvp", bufs=8) as tvp,
            tc.tile_pool(name="psacc", bufs=1, space=bass.MemorySpace.PSUM) as psacc,
            tc.tile_pool(name="pstr", bufs=2, space=bass.MemorySpace.PSUM) as pstr,
        ):
            # ---- constants ----
            # weights go out on the ACT sequencer: it reaches its first
            # instruction ~500ns before SP clears the Tile preamble, and is
            # otherwise idle until the first epilogue (~17us in)
            w0_sb = consts.tile([128, 8, 128], f16)
            nc.scalar.dma_start(out=w0_sb,
                                in_=wt0.rearrange("p (g m) -> p g m", g=8))
            w1_sb = consts.tile([128, NK - 8, 128], f16)
            nc.scalar.dma_start(out=w1_sb,
                                in_=wt1.rearrange("p (g m) -> p g m", g=NK - 8))
            bb_sb = consts.tile([128, 1], f32)
            nc.gpsimd.dma_start(out=bb_sb, in_=bb)
            ident = consts.tile([128, 128], f32)
            make_identity(nc, ident)
            # warm the ACT exp_and_others table during the DMA phase; every
            # later ACT op (Identity/Copy/Exp) stays in this one set.
            warm = consts.tile([1, 1], f32)
            nc.vector.memset(warm, 0.0)
            nc.scalar.add(warm, warm, bb_sb[0:1, :])
            nc.scalar.activation(warm, warm, func=act.Exp)
            # tiles 0-3 stage their results here; one deferred DMA ships
            # them after the last x byte so no output transfer steals
            # mid-stream DMA time
            final_sb = consts.tile([128, 14], f32)

            accs = [psacc.tile([128, 512], f32, tag=f"acc{t}", name=f"acc{t}")
                    for t in range(len(TILES))]

            for t, bt in enumerate(TILES):
                njs = bt // 128
                off = OFFS[t]
                acc = accs[t][:, 0:bt]
                # ---- stream tile t's contraction, accumulate logits.T ----
                # acc[0:64,:] = noise logits.T, acc[64:128,:] = expert
                # logits.T (both pre-bias)
                base = 128 * NK * off
                xtile = xt[base:base + 128 * NK * bt].rearrange(
                    "(p k b) -> p k b", p=128, k=NK)
                k0 = 0
                for gsz in GROUPS[t]:
                    xk = xpool.tile([128, gsz, bt], f16, tag=f"xk{bt}_{gsz}")
                    nc.sync.dma_start(out=xk, in_=xtile[:, k0:k0 + gsz, :])
                    for g in range(gsz):
                        k = k0 + g
                        w = w0_sb[:, k, :] if k < 8 else w1_sb[:, k - 8, :]
                        nc.tensor.matmul(acc, lhsT=w, rhs=xk[:, g, :],
                                         start=(k == 0), stop=(k == NK - 1))
                    k0 += gsz

                # ---- epilogue for tile t (overlaps tile t+1's stream) ----
                # bias-add both halves PSUM->SBUF: noise on ACT, expert on
                # DVE, in parallel
                noiseT = eppool.tile([64, bt], f32, tag=f"nT{bt}")
                nc.scalar.add(noiseT, accs[t][0:64, 0:bt], bb_sb[0:64, :])
                expT = eppool.tile([64, bt], f32, tag=f"eT{bt}")
                nc.vector.tensor_scalar_add(expT, accs[t][64:128, 0:bt],
                                            bb_sb[64:128, :])
                # transpose to batch-major: [128 batch, j | 4+j, 64];
                # noise half first so e_all starts as early as possible
                ps_ne = pstr.tile([128, 8, 64], f32, tag="ps_ne",
                                  name=f"ps_ne{t}")
                for j in range(njs):
                    nc.tensor.transpose(ps_ne[:, j, :],
                                        noiseT[:, j * 128:(j + 1) * 128],
                                        ident[0:64, 0:64])
                for j in range(njs):
                    nc.tensor.transpose(ps_ne[:, 4 + j, :],
                                        expT[:, j * 128:(j + 1) * 128],
                                        ident[0:64, 0:64])
                # softmax numerator without max-shift (|logit| <~ 4); the
                # only readers of ps_ne are the two ACT exps, so the DVE
                # chain below runs entirely from SBUF
                e_all = small.tile([128, 4, 64], f32, tag="e_all")
                nc.scalar.activation(e_all[:, 0:njs, :], ps_ne[:, 0:njs, :],
                                     func=act.Exp)
                eex = small.tile([128, 4, 64], f32, tag="eex")
                nc.scalar.activation(eex[:, 0:njs, :], ps_ne[:, 4:4 + njs, :],
                                     func=act.Exp, scale=-1.0)
                # top-8 on exp(v) (monotone => same selection as on v)
                tvs = []
                for j in range(njs):
                    tv = tvp.tile([128, 8], f32, tag="tv", name=f"tv{t}_{j}")
                    nc.vector.max(tv, e_all[:, j, :])
                    tvs.append(tv)
                # g = e where e >= e8 else 0; zsum = row sum of g
                gts = small.tile([128, 4, 64], f32, tag="gts")
                zsum = small.tile([128, 4], f32, tag="zsum")
                for j in range(njs):
                    nc.vector.scalar_tensor_tensor(
                        out=gts[:, j, :], in0=e_all[:, j, :],
                        scalar=tvs[j][:, 7:8], in1=e_all[:, j, :],
                        op0=alu.is_ge, op1=alu.mult,
                        accum_out=zsum[:, j:j + 1])
                den = small.tile([128, 4, 64], f32, tag="den")
                nc.vector.tensor_scalar_add(den[:, 0:njs, :], eex[:, 0:njs, :],
                                            1.0)
                sig = small.tile([128, 4, 64], f32, tag="sig")
                nc.vector.reciprocal(sig[:, 0:njs, :], den[:, 0:njs, :])
                # s4 = sum_e g*sigmoid
                scr = small.tile([128, 4, 64], f32, tag="scr")
                s4 = small.tile([128, 4], f32, tag="s4")
                for j in range(njs):
                    nc.vector.scalar_tensor_tensor(
                        out=scr[:, j, :], in0=gts[:, j, :], scalar=1.0,
                        in1=sig[:, j, :], op0=alu.mult, op1=alu.mult,
                        accum_out=s4[:, j:j + 1])
                rz = small.tile([128, 4], f32, tag="rz")
                nc.vector.reciprocal(rz[:, 0:njs], zsum[:, 0:njs])
                if t <= 3:
                    c0 = off // 128
                    nc.vector.tensor_mul(final_sb[:, c0:c0 + njs],
                                         s4[:, 0:njs], rz[:, 0:njs])
                    if t == 3:
                        nc.gpsimd.dma_start(
                            out=out[0:1792, :].rearrange(
                                "(j p) o -> p (j o)", j=14, p=128),
                            in_=final_sb)
                else:
                    fin = small.tile([128, 4], f32, tag="fin")
                    nc.vector.tensor_mul(fin[:, 0:njs], s4[:, 0:njs],
                                         rz[:, 0:njs])
                    out_t = out[off:off + bt, :].rearrange(
                        "(j p) o -> p (j o)", j=njs, p=128)
                    eng = nc.sync if t == len(TILES) - 1 else nc.gpsimd
                    eng.dma_start(out=out_t, in_=fin[:, 0:njs])

    nc.compile()
    return nc


def get_program():
    if "prog" not in _cached:
        _cached["prog"] = _build_program()
    return _cached["prog"]


def make_in_maps(x, noise_w, noise_b, expert_w, expert_b):
    """Host-side sharding: per-core transposed fp16 x slice + weights."""
    w_comb = np.concatenate([noise_w, expert_w], axis=0).astype(np.float32)  # [128, D]
    wt32 = np.ascontiguousarray(w_comb.T).astype(np.float16)                 # [D, 128]
    # partition p holds [nk, 128] for contraction rows nk*128+p
    wt = np.ascontiguousarray(
        wt32.reshape(NK, 128, 128).transpose(1, 0, 2).reshape(128, -1))
    wt0 = np.ascontiguousarray(wt[:, :8 * 128])
    wt1 = np.ascontiguousarray(wt[:, 8 * 128:])
    bb = np.concatenate([noise_b, expert_b]).astype(np.float32).reshape(128, 1)
    in_maps = []
    for c in range(NCORES):
        xs = np.ascontiguousarray(x[c * BC:(c + 1) * BC, :].T).astype(np.float16)
        # per tile: [D, bt] -> [128, NK, bt], concatenated flat
        blocks = []
        for t, bt in enumerate(TILES):
            blk = xs[:, OFFS[t]:OFFS[t] + bt].reshape(NK, 128, bt)
            blocks.append(blk.transpose(1, 0, 2).reshape(-1))
        xr = np.ascontiguousarray(np.concatenate(blocks))
        in_maps.append({"xt": xr, "wt0": wt0, "wt1": wt1, "bb": bb})
    return in_maps


def kernel(x, noise, router_w, router_b, noise_w, noise_b, expert_w, expert_b,
           _trace=False):
    from concourse.bass_utils import run_bass_kernel_spmd

    x = np.asarray(x, dtype=np.float32)
    nc = get_program()
    in_maps = make_in_maps(x, np.asarray(noise_w), np.asarray(noise_b),
                           np.asarray(expert_w), np.asarray(expert_b))
    res = run_bass_kernel_spmd(nc, in_maps, core_ids=list(range(NCORES)),
                               trace=_trace)
    out = np.concatenate([r["out"] for r in res.results], axis=0)
    if _trace:
        kernel.last_results = res
    return out


# revision 20
# speedup vs baseline: 1.0008x; 1.0008x over previous
"""MoE logistic regression kernel for 8 Trainium2 NeuronCores.

Math (after dead-code elimination of the reference's unused router path):
    noise_logits = x @ noise_w.T + noise_b            # [B, E]
    top8 = top_k(noise_logits, 8)
    gates = softmax over the top-8 entries (others 0)
    expert = sigmoid(x @ expert_w.T + expert_b)       # [B, E]
    out[b] = sum_e gates[b,e] * expert[b,e]           # [B, 1]

Sharding: batch split 8 ways (2048 rows/core); weights replicated.

Implementation notes:
- Single-pass fp16 matmul (x, w rounded on host). Logit error ~2.3e-4
  flips the 8th/9th expert on ~24/16384 rows; end-to-end l2 rel err
  ~1.2e-3 vs the 2e-2 gate, at half the DMA and a third of the PE work
  of an fp16 hi/lo split. The x stream is the roofline: ~46.6us of
  gapless DMA at the HBM limit.
- Batch-tile-major stream: each tile's full contraction arrives while
  the previous tile's epilogue runs on ACT/DVE. Tile widths taper
  (512,512,512,256,128,128) and the last tile's DMA groups taper too,
  so the serial tail after the last byte is one short epilogue.
- The epilogue never uses the ACT sigmoid table: sigmoid comes from
  exp(-z) + DVE 1/(1+e), and softmax skips the max-shift (logits are
  bounded ~|4|), so every ACT op stays in the one exp_and_others
  function set -- no mid-stream LoadActFuncSet (1.3us each).
- Top-8 gating via DVE Max8 + fused (e >= e8) mask * e with accumulated
  row sum (scalar_tensor_tensor), all on the SBUF exp(v) copy --
  exp is monotone so selection is identical, and avoiding a second
  PSUM reader dodges cross-engine read serialization.
- Per-tile outputs DMA straight from the [128, j] result (no final
  transpose); early tiles go via the idle gpsimd SWDGE path so they
  never head-of-line-block the x stream on the SP HWDGE queue.
"""

import sys

import numpy as np

if "/opt/trn_rl_repo" not in sys.path:
    sys.path.insert(0, "/opt/trn_rl_repo")

B, D, E, TOPK, NCORES = 16384, 4096, 64, 8, 8
BC = B // NCORES      # batch rows per core
NK = D // 128         # contraction chunks
TILES = [512, 512, 512, 256, 128, 128]          # batch tile widths
OFFS = [sum(TILES[:i]) for i in range(len(TILES))]
assert sum(TILES) == BC
# DMA grouping in k-chunks per tile; the final tile tapers so almost no
# matmul work remains after the last byte lands
GROUPS = [[8, 8, 8, 8]] * 5 + [[16, 8, 4, 2, 2]]

_cached = {}


def _build_program():
    import concourse.bass as bass
    import concourse.tile as tile
    from concourse import bacc, mybir
    from concourse.masks import make_identity

    f32 = mybir.dt.float32
    f16 = mybir.dt.float16
    bf16 = mybir.dt.bfloat16
    act = mybir.ActivationFunctionType
    alu = mybir.AluOpType

    nc = bacc.Bacc("TRN2", target_bir_lowering=False, debug=False)
    # x fp16, per-tile partition-major blocks concatenated flat:
    # tile t occupies [128, NK, bt] at element offset 128*NK*OFFS[t], so
    # every group DMA is one contiguous gsz*bt*2-byte run per partition.
    xt = nc.dram_tensor("xt", [NK * 128 * BC], f16, kind="ExternalInput").ap()
    wt0 = nc.dram_tensor("wt0", [128, 8 * 128], f16, kind="ExternalInput").ap()
    wt1 = nc.dram_tensor("wt1", [128, (NK - 8) * 128], f16,
                         kind="ExternalInput").ap()
    bb = nc.dram_tensor("bb", [128, 1], f32, kind="ExternalInput").ap()
    out = nc.dram_tensor("out", [BC, 1], f32, kind="ExternalOutput").ap()

    with tile.TileContext(nc) as tc:
        with (
            tc.tile_pool(name="consts", bufs=1) as consts,
            tc.tile_pool(name="xpool", bufs=6) as xpool,
            tc.tile_pool(name="eppool", bufs=4) as eppool,
            tc.tile_pool(name="small", bufs=2) as small,
            tc.tile_pool(name="tvp", bufs=8) as tvp,
            tc.tile_pool(name="psacc", bufs=1, space=bass.MemorySpace.PSUM) as psacc,
            tc.tile_pool(name="pstr", bufs=2, space=bass.MemorySpace.PSUM) as pstr,
        ):
            # ---- constants ----
            # weights go out on the ACT sequencer: it reaches its first
            # instruction ~500ns before SP clears the Tile preamble, and is
            # otherwise idle until the first epilogue (~17us in)
            w0_sb = consts.tile([128, 8, 128], f16)
            nc.scalar.dma_start(out=w0_sb,
                                in_=wt0.rearrange("p (g m) -> p g m", g=8))
            w1_sb = consts.tile([128, NK - 8, 128], f16)
            nc.scalar.dma_start(out=w1_sb,
                                in_=wt1.rearrange("p (g m) -> p g m", g=NK - 8))
            bb_sb = consts.tile([128, 1], f32)
            nc.gpsimd.dma_start(out=bb_sb, in_=bb)
            ident = consts.tile([128, 128], f32)
            make_identity(nc, ident)
            # warm the ACT exp_and_others table during the DMA phase; every
            # later ACT op (Identity/Copy/Exp) stays in this one set.
            warm = consts.tile([1, 1], f32)
            nc.vector.memset(warm, 0.0)
            nc.scalar.add(warm, warm, bb_sb[0:1, :])
            nc.scalar.activation(warm, warm, func=act.Exp)
            # tiles 0-3 stage their results here; one deferred DMA ships
            # them after the last x byte so no output transfer steals
            # mid-stream DMA time
            final_sb = consts.tile([128, 14], f32)

            accs = [psacc.tile([128, 512], f32, tag=f"acc{t}", name=f"acc{t}")
                    for t in range(len(TILES))]

            for t, bt in enumerate(TILES):
                njs = bt // 128
                off = OFFS[t]
                acc = accs[t][:, 0:bt]
                # ---- stream tile t's contraction, accumulate logits.T ----
                # acc[0:64,:] = noise logits.T, acc[64:128,:] = expert
                # logits.T (both pre-bias)
                base = 128 * NK * off
                xtile = xt[base:base + 128 * NK * bt].rearrange(
                    "(p k b) -> p k b", p=128, k=NK)
                k0 = 0
                for gsz in GROUPS[t]:
                    xk = xpool.tile([128, gsz, bt], f16, tag=f"xk{bt}_{gsz}")
                    nc.sync.dma_start(out=xk, in_=xtile[:, k0:k0 + gsz, :])
                    for g in range(gsz):
                        k = k0 + g
                        w = w0_sb[:, k, :] if k < 8 else w1_sb[:, k - 8, :]
                        nc.tensor.matmul(acc, lhsT=w, rhs=xk[:, g, :],
                                         start=(k == 0), stop=(k == NK - 1))
                    k0 += gsz

                # ---- epilogue for tile t (overlaps tile t+1's stream) ----
                # bias-add both halves PSUM->SBUF: noise on ACT, expert on
                # DVE, in parallel
                noiseT = eppool.tile([64, bt], f32, tag=f"nT{bt}")
                nc.scalar.add(noiseT, accs[t][0:64, 0:bt], bb_sb[0:64, :])
                expT = eppool.tile([64, bt], f32, tag=f"eT{bt}")
                nc.vector.tensor_scalar_add(expT, accs[t][64:128, 0:bt],
                                            bb_sb[64:128, :])
                # transpose to batch-major: [128 batch, j | 4+j, 64];
                # noise half first so e_all starts as early as possible
                ps_ne = pstr.tile([128, 8, 64], f32, tag="ps_ne",
                                  name=f"ps_ne{t}")
                for j in range(njs):
                    nc.tensor.transpose(ps_ne[:, j, :],
                                        noiseT[:, j * 128:(j + 1) * 128],
                                        ident[0:64, 0:64])
                for j in range(njs):
                    nc.tensor.transpose(ps_ne[:, 4 + j, :],
                                        expT[:, j * 128:(j + 1) * 128],
                                        ident[0:64, 0:64])
                # softmax numerator without max-shift (|logit| <~ 4); the
                # only readers of ps_ne are the two ACT exps, so the DVE
                # chain below runs entirely from SBUF
                e_all = small.tile([128, 4, 64], f32, tag="e_all")
                nc.scalar.activation(e_all[:, 0:njs, :], ps_ne[:, 0:njs, :],
                                     func=act.Exp)
                eex = small.tile([128, 4, 64], f32, tag="eex")
                nc.scalar.activation(eex[:, 0:njs, :], ps_ne[:, 4:4 + njs, :],
                                     func=act.Exp, scale=-1.0)
                # top-8 on exp(v) (monotone => same selection as on v)
                tvs = []
                for j in range(njs):
                    tv = tvp.tile([128, 8], f32, tag="tv", name=f"tv{t}_{j}")
                    nc.vector.max(tv, e_all[:, j, :])
                    tvs.append(tv)
                # g = e where e >= e8 else 0; zsum = row sum of g
                gts = small.tile([128, 4, 64], f32, tag="gts")
                zsum = small.tile([128, 4], f32, tag="zsum")
                for j in range(njs):
                    nc.vector.scalar_tensor_tensor(
                        out=gts[:, j, :], in0=e_all[:, j, :],
                        scalar=tvs[j][:, 7:8], in1=e_all[:, j, :],
                        op0=alu.is_ge, op1=alu.mult,
                        accum_out=zsum[:, j:j + 1])
                den = small.tile([128, 4, 64], f32, tag="den")
                nc.vector.tensor_scalar_add(den[:, 0:njs, :], eex[:, 0:njs, :],
                                            1.0)
                sig = small.tile([128, 4, 64], f32, tag="sig")
                nc.vector.reciprocal(sig[:, 0:njs, :], den[:, 0:njs, :])
                # s4 = sum_e g*sigmoid
                scr = small.tile([128, 4, 64], f32, tag="scr")
                s4 = small.tile([128, 4], f32, tag="s4")
                for j in range(njs):
                    nc.vector.scalar_tensor_tensor(
                        out=scr[:, j, :], in0=gts[:, j, :], scalar=1.0,
                        in1=sig[:, j, :], op0=alu.mult, op1=alu.mult,
                        accum_out=s4[:, j:j + 1])
                rz = small.tile([128, 4], f32, tag="rz")
                nc.vector.reciprocal(rz[:, 0:njs], zsum[:, 0:njs])
                if t <= 3:
                    c0 = off // 128
                    nc.vector.tensor_mul(final_sb[:, c0:c0 + njs],
                                         s4[:, 0:njs], rz[:, 0:njs])
                    if t == 3:
                        nc.gpsimd.dma_start(
                            out=out[0:1792, :].rearrange(
                                "(j p) o -> p (j o)", j=14, p=128),
                            in_=final_sb)
                else:
                    fin = small.tile([128, 4], f32, tag="fin")
                    nc.vector.tensor_mul(fin[:, 0:njs], s4[:, 0:njs],
                                         rz[:, 0:njs])
                    out_t = out[off:off + bt, :].rearrange(
                        "(j p) o -> p (j o)", j=njs, p=128)
                    eng = nc.sync if t == len(TILES) - 1 else nc.gpsimd
                    eng.dma_start(out=out_t, in_=fin[:, 0:njs])

    nc.compile()
    return nc


def get_program():
    if "prog" not in _cached:
        _cached["prog"] = _build_program()
    return _cached["prog"]


def make_in_maps(x, noise_w, noise_b, expert_w, expert_b):
    """Host-side sharding: per-core transposed fp16 x slice + weights."""
    w_comb = np.concatenate([noise_w, expert_w], axis=0).astype(np.float32)  # [128, D]
    wt32 = np.ascontiguousarray(w_comb.T).astype(np.float16)                 # [D, 128]
    # partition p holds [nk, 128] for contraction rows nk*128+p
    wt = np.ascontiguousarray(
        wt32.reshape(NK, 128, 128).transpose(1, 0, 2).reshape(128, -1))
    wt0 = np.ascontiguousarray(wt[:, :8 * 128])
    wt1 = np.ascontiguousarray(wt[:, 8 * 128:])
    bb = np.concatenate([noise_b, expert_b]).astype(np.float32).reshape(128, 1)
    in_maps = []
    for c in range(NCORES):
        xs = np.ascontiguousarray(x[c * BC:(c + 1) * BC, :].T).astype(np.float16)
        # per tile: [D, bt] -> [128, NK, bt], concatenated flat
        blocks = []
        for t, bt in enumerate(TILES):
            blk = xs[:, OFFS[t]:OFFS[t] + bt].reshape(NK, 128, bt)
            blocks.append(blk.transpose(1, 0, 2).reshape(-1))
        xr = np.ascontiguousarray(np.concatenate(blocks))
        in_maps.append({"xt": xr, "wt0": wt0, "wt1": wt1, "bb": bb})
    return in_maps


def kernel(x, noise, router_w, router_b, noise_w, noise_b, expert_w, expert_b,
           _trace=False):
    from concourse.bass_utils import run_bass_kernel_spmd

    x = np.asarray(x, dtype=np.float32)
    nc = get_program()
    in_maps = make_in_maps(x, np.asarray(noise_w), np.asarray(noise_b),
                           np.asarray(expert_w), np.asarray(expert_b))
    res = run_bass_kernel_spmd(nc, in_maps, core_ids=list(range(NCORES)),
                               trace=_trace)
    out = np.concatenate([r["out"] for r in res.results], axis=0)
    if _trace:
        kernel.last_results = res
    return out
